# revision 41
# baseline (speedup 1.0000x reference)
"""Multi-head attention (B=2, N=2048, d_model=1024, H=16) on 8 NeuronCores.

Sharding: data-parallel on batch (2) x tensor-parallel on heads (4 groups of
4 heads). Core c handles batch c//4, head-group c%4: its heads' Q/K/V
projections, causal attention, and a partial output projection; the host
sums the 4 partials per batch.

v2 pipeline (per core):
- Q/K projections run in fp8e4 (x and 32x-scaled weights quantized on host)
  using DoubleRow perf-mode matmuls: two 128-deep k-tiles per instruction at
  0.5 cycles/row -> 4x cheaper than bf16. Scores only pass through softmax,
  which tolerates the quantization. V projection, S, PV and the output
  projection stay bf16 (fp8 on the value path costs too much accuracy).
- S is computed per head in S^T orientation [keys, queries], bf16.
- exp: off-diagonal blocks on ACT (exact exp -> bf16, with the softmax
  shift exp(s*scale - 1.5) for range safety). Diagonal blocks on DVE via a
  Schraudolph int16 bit-trick fused with the causal mask: a per-block bias
  tile B holds the magic constant on valid positions and -1e9 above the
  diagonal, so one scalar_tensor_tensor emits masked bf16 weights directly
  (saturating int16 -> bitcast bf16; -32768 bitcasts to -0.0).
- PV runs transposed: stationary = pT [128 keys, 128 queries], moving =
  v-tile [128 keys, 66] (64 dims + ones column for the denominator + pad),
  accumulating [128 queries, 66] per (head, query-block). Cost per matmul is
  the 66-wide moving size, which roughly halves PV cost vs the classic
  orientation, and it makes the softmax denominator a per-partition scalar:
  reciprocal of a [128,4] column + one fused multiply normalizes a whole
  unit. PE transposes (via identity matmul) bring xa back to [dhc, queries]
  for the bf16 output projection.
"""

import sys

if "/opt/trn_rl_repo" not in sys.path:
    sys.path.insert(0, "/opt/trn_rl_repo")

import numpy as np
import ml_dtypes

import concourse.bass as bass
import concourse.mybir as mybir
import concourse.tile as tile
from concourse import bacc
from concourse.bass_utils import run_bass_kernel_spmd

B, N, D, H = 2, 2048, 1024, 16
DV = D // H  # 64
HPC = H // 4  # heads per core: 4
DHC = HPC * DV  # head dims per core: 256
NT = N // 128  # 16 key blocks
NC = N // 512  # 4 query chunks
DT = D // 128  # 8 d_in tiles
VW = 66  # PV moving width: 64 dims + ones col + pad
BF = mybir.dt.bfloat16
F32 = mybir.dt.float32
FP8 = mybir.dt.float8e4
F16 = mybir.dt.float16
I16 = mybir.dt.int16
EXP = mybir.ActivationFunctionType.Exp
DRM = mybir.MatmulPerfMode.DoubleRow
MUL = mybir.AluOpType.mult
ADD = mybir.AluOpType.add

LOG2E = float(np.log2(np.e))
SEXP = 0.125 / 1024.0  # exp scale on raw scores (32x-scaled q and k)
CSH = 1.5  # softmax shift: exp(s*SEXP - CSH)
CFIT = -8.0  # schraudolph magic correction (fit in numpy)
A16 = 128.0 * LOG2E * SEXP
M16 = 128.0 * 127.0 + CFIT - 128.0 * LOG2E * CSH

_CACHE = {}


def build_nc():
    nc = bacc.Bacc("TRN2", target_bir_lowering=False, debug=False)
    xqT8_d = nc.dram_tensor("xqT8", [D, N], FP8, kind="ExternalInput")
    xkT8_d = nc.dram_tensor("xkT8", [D, N], FP8, kind="ExternalInput")
    xv8_d = nc.dram_tensor("xv8", [128, 4, NT, 2, 128], FP8, kind="ExternalInput")
    xvr8_d = nc.dram_tensor(
        "xvr8", [128, 4, NT, 2, 128], mybir.dt.float8e5, kind="ExternalInput"
    )
    wv8_d = nc.dram_tensor("wv8", [128, 4, 2, DHC], FP8, kind="ExternalInput")
    wvr8_d = nc.dram_tensor(
        "wvr8", [128, 4, 2, DHC], mybir.dt.float8e5, kind="ExternalInput"
    )
    # qk weights, 32x scale, fp8, host-arranged [p, j2, part, i, col]
    wq8_d = nc.dram_tensor("wq8", [128, 4, 2, 2, 128], FP8, kind="ExternalInput")
    woT_d = nc.dram_tensor("woT", [DHC, D], BF, kind="ExternalInput")
    bq32_d = nc.dram_tensor("bq32", [DHC], F32, kind="ExternalInput")
    bq1_d = nc.dram_tensor("bq1", [DHC], F32, kind="ExternalInput")
    b16_d = nc.dram_tensor("b16", [128, 512], F32, kind="ExternalInput")
    ident_d = nc.dram_tensor("ident", [128, 128], BF, kind="ExternalInput")
    yT_d = nc.dram_tensor("yT", [D, N], F16, kind="ExternalOutput")

    with tile.TileContext(nc) as tc:
        with (
            tc.tile_pool(name="consts", bufs=1) as consts,
            tc.tile_pool(name="xin", bufs=1) as xin,
            tc.tile_pool(name="prod", bufs=1) as prod,
            tc.tile_pool(name="work", bufs=3) as work,
            tc.tile_pool(name="norm", bufs=3) as norm,
            tc.tile_pool(name="yout", bufs=2) as yout,
            tc.tile_pool(name="ps", bufs=1, space="PSUM") as ps,
        ):
            # ---- weights + constants ----
            wq8 = consts.tile([128, 4, 2, 2, 128], FP8, name="wq8")
            nc.sync.dma_start(out=wq8, in_=wq8_d.ap())
            # ---- bulk inputs ----
            xkT8 = xin.tile([128, DT, N], FP8, name="xkT8")
            xqT8 = xin.tile([128, DT, N], FP8, name="xqT8")
            xv8 = xin.tile([128, 4, NT, 2, 128], FP8, name="xv8")
            xvr8 = xin.tile(
                [128, 4, NT, 2, 128], mybir.dt.float8e5, name="xvr8"
            )

            def load_slice(t, d, n0, n1):
                nc.sync.dma_start(
                    out=t[:, :, n0:n1],
                    in_=d.ap()[:, n0:n1].rearrange("(j p) n -> p j n", p=128),
                )

            def load_j2(t, d, j2):
                nc.sync.dma_start(
                    out=t[:, j2 : j2 + 2, :],
                    in_=d.ap()[j2 * 128 : (j2 + 2) * 128, :].rearrange(
                        "(j p) n -> p j n", p=128
                    ),
                )

            nc.sync.dma_start(
                out=xkT8[:, 0:2, 0:512],
                in_=xkT8_d.ap()[0:256, 0:512].rearrange(
                    "(j p) n -> p j n", p=128
                ),
            )
            nc.sync.dma_start(
                out=xkT8[:, 2:8, 0:512],
                in_=xkT8_d.ap()[256:1024, 0:512].rearrange(
                    "(j p) n -> p j n", p=128
                ),
            )
            bq_pp = consts.tile([128, 2], F32, name="bq_pp")
            nc.sync.dma_start(
                out=bq_pp, in_=bq32_d.ap().rearrange("(c p) -> p c", p=128)
            )
            load_slice(xqT8, xqT8_d, 0, 512)
            b16 = consts.tile([128, 512], F32, name="b16")
            nc.sync.dma_start(out=b16, in_=b16_d.ap())
            bq_row = consts.tile([1, DHC], F32, name="bq_row")
            nc.sync.dma_start(
                out=bq_row, in_=bq1_d.ap().rearrange("(a c) -> a c", a=1)
            )
            bq_bc = consts.tile([128, DHC], F32, name="bq_bc")
            nc.gpsimd.partition_broadcast(bq_bc, bq_row)
            wv8 = consts.tile([128, 4, 2, DHC], FP8, name="wv8")
            nc.sync.dma_start(out=wv8, in_=wv8_d.ap())
            wvr8 = consts.tile([128, 4, 2, DHC], mybir.dt.float8e5, name="wvr8")
            nc.sync.dma_start(out=wvr8, in_=wvr8_d.ap())

            def load_v(mc):
                nc.sync.dma_start(
                    out=xv8[:, :, 4 * mc : 4 * mc + 4, :, :],
                    in_=xv8_d.ap()[:, :, 4 * mc : 4 * mc + 4, :, :],
                )
                nc.sync.dma_start(
                    out=xvr8[:, :, 4 * mc : 4 * mc + 4, :, :],
                    in_=xvr8_d.ap()[:, :, 4 * mc : 4 * mc + 4, :, :],
                )

            load_v(0)
            load_slice(xkT8, xkT8_d, 512, 1024)
            load_slice(xqT8, xqT8_d, 512, 1024)
            ident = consts.tile([128, 128], BF, name="ident")
            nc.sync.dma_start(out=ident, in_=ident_d.ap())
            load_v(1)
            load_slice(xkT8, xkT8_d, 1536, 2048)
            load_slice(xqT8, xqT8_d, 1536, 2048)
            load_slice(xkT8, xkT8_d, 1024, 1536)
            load_slice(xqT8, xqT8_d, 1024, 1536)
            load_v(2)
            load_v(3)
            negc = consts.tile([128, 1], F32, name="negc")
            nc.vector.memset(negc, -CSH)
            woT = consts.tile([128, 2, D], BF, name="woT")
            nc.sync.dma_start(
                out=woT, in_=woT_d.ap().rearrange("(q p) c -> p q c", p=128)
            )

            # ---- persistent produce tiles ----
            qT = [prod.tile([128, N], BF, name=f"qT{p}") for p in range(2)]
            kT = [prod.tile([128, N], BF, name=f"kT{p}") for p in range(2)]
            xaT = [prod.tile([128, N], BF, name=f"xaT{p}") for p in range(2)]
            vpp = prod.tile([128, NT, HPC, VW], BF, name="vpp")
            nc.vector.memset(vpp[:, :, :, DV : DV + 1], 1.0)
            nc.vector.memset(vpp[:, :, :, DV + 1 : VW], 0.0)

            # ---- building blocks ----
            def proj_qk(src8, dst, c, p):
                """q/k projection for part p, chunk c: 4 fp8 DoubleRow
                matmuls (256-deep each), then PSUM->bf16 copy w/ bias."""
                pp = ps.tile([128, 512], F32, name="prj_qk", tag="prj", bufs=2)
                for j2 in range(4):
                    nc.tensor.matmul(
                        pp,
                        wq8[:, j2, p, :, :],
                        src8[:, 2 * j2 : 2 * j2 + 2, c * 512 : (c + 1) * 512],
                        start=(j2 == 0),
                        stop=(j2 == 3),
                        perf_mode=DRM,
                    )
                nc.scalar.activation(
                    dst[p][:, c * 512 : (c + 1) * 512],
                    pp,
                    mybir.ActivationFunctionType.Identity,
                    bias=bq_pp[:, p : p + 1],
                )

            def proj_v(m):
                """v projection for key block m: fp8 DoubleRow with e5m2
                error-feedback residuals on both operands (3 chains:
                x8*w8 + x8*wr + rx*w8) reconstructs the product to ~bf16
                accuracy at fp8-DR speed. Weights carry a 32x scale,
                removed in the bias-add STT."""
                pv = ps.tile([128, 512], F32, name="prj_v", tag="prj", bufs=2)
                pvv = pv[:, 0:DHC]
                for j2 in range(4):
                    nc.tensor.matmul(
                        pvv, xv8[:, j2, m, :, :], wv8[:, j2, :, :],
                        start=(j2 == 0), stop=False, perf_mode=DRM,
                    )
                for j2 in range(4):
                    nc.tensor.matmul(
                        pvv, xv8[:, j2, m, :, :], wvr8[:, j2, :, :],
                        start=False, stop=False, perf_mode=DRM,
                    )
                for j2 in range(4):
                    nc.tensor.matmul(
                        pvv, xvr8[:, j2, m, :, :], wv8[:, j2, :, :],
                        start=False, stop=(j2 == 3), perf_mode=DRM,
                    )
                nc.vector.scalar_tensor_tensor(
                    out=vpp[:, m, :, 0:DV],
                    in0=pvv.rearrange("p (h d) -> p h d", h=HPC),
                    scalar=1.0 / 32.0,
                    in1=bq_bc.rearrange("p (h d) -> p h d", h=HPC),
                    op0=MUL,
                    op1=ADD,
                )

            y_stage = {}

            def outproj_t(c, t, act_copy=False):
                yp = ps.tile([128, 512], F32, name="yp", tag="prj", bufs=2)
                for p in range(2):
                    nc.tensor.matmul(
                        yp,
                        woT[:, p, t * 128 : (t + 1) * 128],
                        xaT[p][:, c * 512 : (c + 1) * 512],
                        start=(p == 0),
                        stop=(p == 1),
                    )
                if t == 0:
                    y_stage[c] = yout.tile(
                        [128, DT, 512], F16, name="y_sb", tag="ystage",
                        bufs=2,
                    )
                y_sb = y_stage[c]
                if act_copy:
                    nc.scalar.copy(y_sb[:, t, :], yp)
                else:
                    nc.vector.tensor_copy(y_sb[:, t, :], yp)
                if c == 2:
                    nc.sync.dma_start(
                        out=yT_d.ap()[
                            t * 128 : (t + 1) * 128,
                            c * 512 : (c + 1) * 512,
                        ].rearrange("(t p) n -> p t n", p=128),
                        in_=y_sb[:, t : t + 1, :],
                    )
                elif t % 2 == 1:
                    t0 = t - 1
                    nc.sync.dma_start(
                        out=yT_d.ap()[
                            t0 * 128 : (t0 + 2) * 128,
                            c * 512 : (c + 1) * 512,
                        ].rearrange("(t p) n -> p t n", p=128),
                        in_=y_sb[:, t0 : t0 + 2, :],
                    )

            pT_tiles = {}
            op_tiles = {}
            xan_tiles = {}
            tp_tiles = {}

            def s_exp_burst(c, h, fillers):
                """S matmuls + exp for unit (c, h). Diagonal blocks:
                DVE STT schraudolph with mask folded into the B tile.
                Full blocks: ACT exact exp."""
                hp, hr = h // 2, h % 2
                fi = list(fillers)
                for j in range(4 * c + 4):
                    off = max(0, (j - 4 * c) * 128)
                    w = 512 - off
                    sp = ps.tile([128, 512], F32, name="sp", tag="spx", bufs=5)
                    pT = work.tile(
                        [128, 512], BF, name="pT", tag="pT", bufs=70
                    )
                    nc.tensor.matmul(
                        sp[:, 0:w],
                        kT[hp][
                            hr * 64 : (hr + 1) * 64, j * 128 : (j + 1) * 128
                        ],
                        qT[hp][
                            hr * 64 : (hr + 1) * 64,
                            c * 512 + off : (c + 1) * 512,
                        ],
                        start=True,
                        stop=True,
                    )
                    if off or j >= 4 * c:
                        # diagonal block: schraudolph + causal mask via B
                        nc.vector.scalar_tensor_tensor(
                            out=pT.bitcast(I16)[:, off:512],
                            in0=sp[:, 0:w],
                            scalar=A16,
                            in1=b16[:, 0:w],
                            op0=MUL,
                            op1=ADD,
                        )
                    elif (j % 4 == 3 and not (c == 2 and h >= 2)) or (
                        c == 3 and j % 4 == 1
                    ):
                        # every 4th full block: schraudolph on DVE to
                        # offload the ACT exp stream
                        nc.vector.tensor_scalar(
                            pT.bitcast(I16), sp, A16, M16, MUL, ADD
                        )
                    else:
                        nc.scalar.activation(
                            pT, sp, EXP, scale=SEXP, bias=negc
                        )
                    pT_tiles[(c, h, j)] = pT
                    if fi:
                        fi.pop(0)()
                for f in fi:
                    f()

            def pv_seg(c, h, t):
                """One query-block segment of transposed PV for unit
                (c, h): accumulate [128 queries, VW] chained over key
                blocks j <= 4c+t. Moving is the 66-wide v tile."""
                if t == 0:
                    op_tiles[(c, h)] = ps.tile(
                        [128, 4, VW], F32, name="op", tag="op", bufs=1
                    )
                op = op_tiles[(c, h)]
                qb = 4 * c + t
                qoff = t * 128
                for j in range(qb + 1):
                    pT = pT_tiles[(c, h, j)]
                    nc.tensor.matmul(
                        op[:, t, :],
                        pT[:, qoff : qoff + 128],
                        vpp[:, j, h, :],
                        start=(t == 0 and j == 0),
                        stop=(t == 3 and j == qb),
                        skip_group_check=True,
                    )
                if t == 3:
                    for j in range(4 * c + 4):
                        del pT_tiles[(c, h, j)]

            def norm_unit(c, h):
                """Normalize unit (c, h): per-partition reciprocal of the
                ones-column, one fused multiply."""
                op = op_tiles.pop((c, h))
                rcp = norm.tile([128, 4], F32, name="rcp", tag="rcp")
                nc.vector.reciprocal(rcp, op[:, :, DV])
                xan = norm.tile([128, 4, DV], BF, name="xan", tag="xan")
                nc.vector.scalar_tensor_tensor(
                    out=xan,
                    in0=op[:, :, 0:DV],
                    scalar=1.0,
                    in1=rcp.unsqueeze(2).broadcast_to([128, 4, DV]),
                    op0=MUL,
                    op1=MUL,
                )
                xan_tiles[(c, h)] = xan

            def transpose_xa(c, h):
                """PE-transpose xa [queries, dv] -> [dv, queries] into the
                shared tp psum tile; after the second head of a pair, copy
                the [128, 512] block out to xaT."""
                hp, hr = h // 2, h % 2
                xan = xan_tiles.pop((c, h))
                if hr == 0:
                    tp = ps.tile([128, 1024], BF, name="tp", tag="spx", bufs=5)
                    tp_tiles[(c, hp)] = tp
                tp = tp_tiles[(c, hp)]
                for t in range(4):
                    nc.tensor.matmul(
                        tp[hr * 64 : (hr + 1) * 64, t * 128 : (t + 1) * 128],
                        xan[:, t, :],
                        ident,
                        start=(hr == 0 and t == 0),
                        stop=(hr == 1 and t == 3),
                        is_transpose=True,
                        skip_group_check=True,
                    )
                if hr == 1:
                    del tp_tiles[(c, hp)]
                    nc.vector.tensor_copy(
                        xaT[hp][:, c * 512 : (c + 1) * 512], tp[:, 0:512]
                    )

            def F(fn, *a):
                return lambda: fn(*a)

            # ---- schedule: heavy (c=3/2) and light (c=0/1) units
            # interleaved so the exp engines' backlog from a heavy burst
            # drains while PE runs a light one ----
            UNITS = [
                (0, 0), (0, 1), (0, 2), (0, 3), (1, 0), (1, 1), (1, 2),
                (1, 3), (3, 0), (3, 1), (3, 2), (3, 3), (2, 0), (2, 1),
                (2, 2), (2, 3),
            ]
            fillers = {u: [] for u in UNITS}
            pre = {u: [] for u in UNITS}
            pre[(1, 0)] = [
                F(proj_qk, xkT8, kT, 1, 0),
                F(proj_qk, xkT8, kT, 1, 1),
                F(proj_qk, xqT8, qT, 1, 0),
                F(proj_qk, xqT8, qT, 1, 1),
            ]
            fillers[(0, 2)] = [F(proj_v, 0), F(proj_v, 1)]
            pre[(0, 3)] = [F(proj_v, 2), F(proj_v, 3)]
            fillers[(1, 2)] = [F(proj_v, m) for m in range(4, 8)]
            pre[(3, 0)] = [
                F(proj_qk, xqT8, qT, 3, 0),
                F(proj_qk, xqT8, qT, 3, 1),
                F(proj_qk, xkT8, kT, 3, 0),
                F(proj_qk, xkT8, kT, 3, 1),
            ]
            fillers[(3, 0)] = [
                F(proj_qk, xkT8, kT, 2, 0),
                F(proj_qk, xkT8, kT, 2, 1),
            ]
            fillers[(3, 1)] = [F(proj_v, m) for m in range(12, 16)] + [
                F(outproj_t, 0, t, bool(t % 2)) for t in range(4)
            ]
            fillers[(3, 2)] = [F(outproj_t, 0, t, bool(t % 2)) for t in range(4, 8)] + [
                F(proj_v, m) for m in range(8, 12)
            ]
            fillers[(3, 3)] = [
                F(proj_qk, xqT8, qT, 2, 0),
                F(proj_qk, xqT8, qT, 2, 1),
            ]
            fillers[(2, 0)] = [F(outproj_t, 1, t, bool(t % 2)) for t in range(4)]
            fillers[(2, 1)] = [F(outproj_t, 1, t, bool(t % 2)) for t in range(4, 8)]
            fillers[(2, 3)] = [F(outproj_t, 3, t, bool(t % 2)) for t in range(8)]

            # prologue: chunk-0 q/k projections
            for p in range(2):
                proj_qk(xkT8, kT, 0, p)
            for p in range(2):
                proj_qk(xqT8, qT, 0, p)

            for i, u in enumerate(UNITS):
                for f in pre[u]:
                    f()
                q = []
                if i >= 3:
                    u2 = UNITS[i - 3]
                    q += [F(pv_seg, *u2, t) for t in range(2)]
                if i >= 4:
                    q.append(F(transpose_xa, *UNITS[i - 4]))
                if i >= 3:
                    u2 = UNITS[i - 3]
                    q += [F(pv_seg, *u2, t) for t in range(2, 4)]
                    q.append(F(norm_unit, *u2))
                if i == 15:
                    u2 = UNITS[13]
                    q += [F(pv_seg, *u2, t) for t in range(4)]
                    q.append(F(norm_unit, *u2))
                q += fillers[u]
                s_exp_burst(*u, q)
            for i in (14, 15):
                u2 = UNITS[i]
                for t in range(4):
                    pv_seg(*u2, t)
                norm_unit(*u2)
            for i in (12, 13, 14, 15):
                transpose_xa(*UNITS[i])
            for t in range(DT):
                outproj_t(2, t, act_copy=bool(t % 2 == 0))
    nc.compile()
    return nc


def kernel(**inputs):
    inputs = {k: np.asarray(v) for k, v in inputs.items()}
    Q, K, V = inputs["Q"], inputs["K"], inputs["V"]
    wq, bq, wo, bo = inputs["wq"], inputs["bq"], inputs["wo"], inputs["bo"]
    f8 = ml_dtypes.float8_e4m3
    b16t = ml_dtypes.bfloat16

    def f8T(x):  # fp8 transpose [n, d] -> [d, n]
        return np.ascontiguousarray(x.astype(f8).T)

    def bfT(x):
        return np.ascontiguousarray(x.astype(b16t).T)

    xqT8 = [f8T(Q[b]) for b in range(B)]
    xkT8 = [f8T(K[b]) for b in range(B)]
    f8e5 = ml_dtypes.float8_e5m2
    xv8s, xvr8s = [], []
    for b in range(B):
        xvg = np.ascontiguousarray(V[b].T.astype(np.float32))  # [D, N]
        arr = xvg.reshape(4, 2, 128, NT, 128).transpose(2, 0, 3, 1, 4)
        a8 = arr.astype(f8)
        xv8s.append(np.ascontiguousarray(a8))
        xvr8s.append(
            np.ascontiguousarray((arr - a8.astype(np.float32)).astype(f8e5))
        )

    wq8s, wv8s, wvr8s, woTs, bq32s, bq1s = [], [], [], [], [], []
    for g in range(4):
        wqg = wq[g * DHC : (g + 1) * DHC, :]  # [256, 1024]
        # [p, j2, part, i, col]: value = 32*wq[part*128+col, (2*j2+i)*128+p]
        t = (32.0 * wqg).reshape(2, 128, 4, 2, 128)  # [part, col, j2, i, p]
        wq8s.append(
            np.ascontiguousarray(t.transpose(4, 2, 0, 3, 1).astype(f8))
        )
        wv = (32.0 * wqg).T.astype(np.float32)  # [D, 256]
        warr = wv.reshape(4, 2, 128, DHC).transpose(2, 0, 1, 3)
        w8v = warr.astype(f8)
        wv8s.append(np.ascontiguousarray(w8v))
        wvr8s.append(
            np.ascontiguousarray((warr - w8v.astype(np.float32)).astype(f8e5))
        )
        woTs.append(bfT(wo[:, g * DHC : (g + 1) * DHC]))
        bq32s.append(
            np.ascontiguousarray(
                32.0 * bq[g * DHC : (g + 1) * DHC], dtype=np.float32
            )
        )
        bq1s.append(
            np.ascontiguousarray(bq[g * DHC : (g + 1) * DHC], dtype=np.float32)
        )

    r = np.arange(128)[:, None]
    u = np.arange(512)[None, :]
    b16m = np.where(u >= r, np.float32(M16), np.float32(-1e9)).astype(
        np.float32
    )
    ident = np.eye(128, dtype=b16t)

    if "nc" not in _CACHE:
        _CACHE["nc"] = build_nc()
    nc = _CACHE["nc"]

    in_maps = []
    for core in range(8):
        b, g = divmod(core, 4)
        in_maps.append(
            {
                "xqT8": xqT8[b],
                "xkT8": xkT8[b],
                "xv8": xv8s[b],
                "xvr8": xvr8s[b],
                "wq8": wq8s[g],
                "wv8": wv8s[g],
                "wvr8": wvr8s[g],
                "woT": woTs[g],
                "bq32": bq32s[g],
                "bq1": bq1s[g],
                "b16": b16m,
                "ident": ident,
            }
        )
    import os

    trace = bool(int(os.environ.get("KERNEL_TRACE", "0")))
    try:
        res = run_bass_kernel_spmd(
            nc, in_maps, core_ids=list(range(8)), trace=trace
        )
    except ModuleNotFoundError:
        res = run_bass_kernel_spmd(nc, in_maps, core_ids=list(range(8)))
    _CACHE["last_results"] = res

    out = np.empty((B, N, D), np.float32)
    for b in range(B):
        acc = res.results[4 * b]["yT"].astype(np.float32)
        for g in range(1, 4):
            acc += res.results[4 * b + g]["yT"]
        out[b] = acc.T + bo
    return out


# revision 42
# speedup vs baseline: 1.0035x; 1.0035x over previous
"""Multi-head attention (B=2, N=2048, d_model=1024, H=16) on 8 NeuronCores.

Sharding: data-parallel on batch (2) x tensor-parallel on heads (4 groups of
4 heads). Core c handles batch c//4, head-group c%4: its heads' Q/K/V
projections, causal attention, and a partial output projection; the host
sums the 4 partials per batch.

v2 pipeline (per core):
- Q/K projections run in fp8e4 (x and 32x-scaled weights quantized on host)
  using DoubleRow perf-mode matmuls: two 128-deep k-tiles per instruction at
  0.5 cycles/row -> 4x cheaper than bf16. Scores only pass through softmax,
  which tolerates the quantization. V projection, S, PV and the output
  projection stay bf16 (fp8 on the value path costs too much accuracy).
- S is computed per head in S^T orientation [keys, queries], bf16.
- exp: off-diagonal blocks on ACT (exact exp -> bf16, with the softmax
  shift exp(s*scale - 1.5) for range safety). Diagonal blocks on DVE via a
  Schraudolph int16 bit-trick fused with the causal mask: a per-block bias
  tile B holds the magic constant on valid positions and -1e9 above the
  diagonal, so one scalar_tensor_tensor emits masked bf16 weights directly
  (saturating int16 -> bitcast bf16; -32768 bitcasts to -0.0).
- PV runs transposed: stationary = pT [128 keys, 128 queries], moving =
  v-tile [128 keys, 66] (64 dims + ones column for the denominator + pad),
  accumulating [128 queries, 66] per (head, query-block). Cost per matmul is
  the 66-wide moving size, which roughly halves PV cost vs the classic
  orientation, and it makes the softmax denominator a per-partition scalar:
  reciprocal of a [128,4] column + one fused multiply normalizes a whole
  unit. PE transposes (via identity matmul) bring xa back to [dhc, queries]
  for the bf16 output projection.
"""

import sys

if "/opt/trn_rl_repo" not in sys.path:
    sys.path.insert(0, "/opt/trn_rl_repo")

import numpy as np
import ml_dtypes

import concourse.bass as bass
import concourse.mybir as mybir
import concourse.tile as tile
from concourse import bacc
from concourse.bass_utils import run_bass_kernel_spmd

B, N, D, H = 2, 2048, 1024, 16
DV = D // H  # 64
HPC = H // 4  # heads per core: 4
DHC = HPC * DV  # head dims per core: 256
NT = N // 128  # 16 key blocks
NC = N // 512  # 4 query chunks
DT = D // 128  # 8 d_in tiles
VW = 66  # PV moving width: 64 dims + ones col + pad
BF = mybir.dt.bfloat16
F32 = mybir.dt.float32
FP8 = mybir.dt.float8e4
F16 = mybir.dt.float16
I16 = mybir.dt.int16
EXP = mybir.ActivationFunctionType.Exp
DRM = mybir.MatmulPerfMode.DoubleRow
MUL = mybir.AluOpType.mult
ADD = mybir.AluOpType.add

LOG2E = float(np.log2(np.e))
SEXP = 0.125 / 1024.0  # exp scale on raw scores (32x-scaled q and k)
CSH = 1.5  # softmax shift: exp(s*SEXP - CSH)
CFIT = -8.0  # schraudolph magic correction (fit in numpy)
A16 = 128.0 * LOG2E * SEXP
M16 = 128.0 * 127.0 + CFIT - 128.0 * LOG2E * CSH

_CACHE = {}


def build_nc():
    nc = bacc.Bacc("TRN2", target_bir_lowering=False, debug=False)
    xqT8_d = nc.dram_tensor("xqT8", [D, N], FP8, kind="ExternalInput")
    xkT8_d = nc.dram_tensor("xkT8", [D, N], FP8, kind="ExternalInput")
    xv8_d = nc.dram_tensor("xv8", [128, 4, NT, 2, 128], FP8, kind="ExternalInput")
    xvr8_d = nc.dram_tensor(
        "xvr8", [128, 4, NT, 2, 128], mybir.dt.float8e5, kind="ExternalInput"
    )
    wv8_d = nc.dram_tensor("wv8", [128, 4, 2, DHC], FP8, kind="ExternalInput")
    wvr8_d = nc.dram_tensor(
        "wvr8", [128, 4, 2, DHC], mybir.dt.float8e5, kind="ExternalInput"
    )
    # qk weights, 32x scale, fp8, host-arranged [p, j2, part, i, col]
    wq8_d = nc.dram_tensor("wq8", [128, 4, 2, 2, 128], FP8, kind="ExternalInput")
    woT_d = nc.dram_tensor("woT", [DHC, D], BF, kind="ExternalInput")
    bq32_d = nc.dram_tensor("bq32", [DHC], F32, kind="ExternalInput")
    bq1_d = nc.dram_tensor("bq1", [DHC], F32, kind="ExternalInput")
    b16_d = nc.dram_tensor("b16", [128, 512], F32, kind="ExternalInput")
    ident_d = nc.dram_tensor("ident", [128, 128], BF, kind="ExternalInput")
    yT_d = nc.dram_tensor("yT", [D, N], F16, kind="ExternalOutput")

    with tile.TileContext(nc) as tc:
        with (
            tc.tile_pool(name="consts", bufs=1) as consts,
            tc.tile_pool(name="xin", bufs=1) as xin,
            tc.tile_pool(name="prod", bufs=1) as prod,
            tc.tile_pool(name="work", bufs=3) as work,
            tc.tile_pool(name="norm", bufs=3) as norm,
            tc.tile_pool(name="yout", bufs=2) as yout,
            tc.tile_pool(name="ps", bufs=1, space="PSUM") as ps,
        ):
            # ---- weights + constants ----
            wq8 = consts.tile([128, 4, 2, 2, 128], FP8, name="wq8")
            nc.sync.dma_start(out=wq8, in_=wq8_d.ap())
            # ---- bulk inputs ----
            xkT8 = xin.tile([128, DT, N], FP8, name="xkT8")
            xqT8 = xin.tile([128, DT, N], FP8, name="xqT8")
            xv8 = xin.tile([128, 4, NT, 2, 128], FP8, name="xv8")
            xvr8 = xin.tile(
                [128, 4, NT, 2, 128], mybir.dt.float8e5, name="xvr8"
            )

            def load_slice(t, d, n0, n1):
                nc.sync.dma_start(
                    out=t[:, :, n0:n1],
                    in_=d.ap()[:, n0:n1].rearrange("(j p) n -> p j n", p=128),
                )

            def load_j2(t, d, j2):
                nc.sync.dma_start(
                    out=t[:, j2 : j2 + 2, :],
                    in_=d.ap()[j2 * 128 : (j2 + 2) * 128, :].rearrange(
                        "(j p) n -> p j n", p=128
                    ),
                )

            load_slice(xkT8, xkT8_d, 0, 512)
            bq_pp = consts.tile([128, 2], F32, name="bq_pp")
            nc.sync.dma_start(
                out=bq_pp, in_=bq32_d.ap().rearrange("(c p) -> p c", p=128)
            )
            load_slice(xqT8, xqT8_d, 0, 512)
            b16 = consts.tile([128, 512], F32, name="b16")
            nc.sync.dma_start(out=b16, in_=b16_d.ap())
            bq_row = consts.tile([1, DHC], F32, name="bq_row")
            nc.sync.dma_start(
                out=bq_row, in_=bq1_d.ap().rearrange("(a c) -> a c", a=1)
            )
            bq_bc = consts.tile([128, DHC], F32, name="bq_bc")
            nc.gpsimd.partition_broadcast(bq_bc, bq_row)
            wv8 = consts.tile([128, 4, 2, DHC], FP8, name="wv8")
            nc.sync.dma_start(out=wv8, in_=wv8_d.ap())
            wvr8 = consts.tile([128, 4, 2, DHC], mybir.dt.float8e5, name="wvr8")
            nc.sync.dma_start(out=wvr8, in_=wvr8_d.ap())

            def load_v(mc):
                nc.sync.dma_start(
                    out=xv8[:, :, 4 * mc : 4 * mc + 4, :, :],
                    in_=xv8_d.ap()[:, :, 4 * mc : 4 * mc + 4, :, :],
                )
                nc.sync.dma_start(
                    out=xvr8[:, :, 4 * mc : 4 * mc + 4, :, :],
                    in_=xvr8_d.ap()[:, :, 4 * mc : 4 * mc + 4, :, :],
                )

            load_v(0)
            load_slice(xkT8, xkT8_d, 512, 1024)
            load_slice(xqT8, xqT8_d, 512, 1024)
            ident = consts.tile([128, 128], BF, name="ident")
            nc.sync.dma_start(out=ident, in_=ident_d.ap())
            load_v(1)
            load_slice(xkT8, xkT8_d, 1536, 2048)
            load_slice(xqT8, xqT8_d, 1536, 2048)
            load_slice(xkT8, xkT8_d, 1024, 1536)
            load_slice(xqT8, xqT8_d, 1024, 1536)
            load_v(2)
            load_v(3)
            negc = consts.tile([128, 1], F32, name="negc")
            nc.vector.memset(negc, -CSH)
            woT = consts.tile([128, 2, D], BF, name="woT")
            nc.sync.dma_start(
                out=woT, in_=woT_d.ap().rearrange("(q p) c -> p q c", p=128)
            )

            # ---- persistent produce tiles ----
            qT = [prod.tile([128, N], BF, name=f"qT{p}") for p in range(2)]
            kT = [prod.tile([128, N], BF, name=f"kT{p}") for p in range(2)]
            xaT = [prod.tile([128, N], BF, name=f"xaT{p}") for p in range(2)]
            vpp = prod.tile([128, NT, HPC, VW], BF, name="vpp")
            nc.vector.memset(vpp[:, :, :, DV : DV + 1], 1.0)
            nc.vector.memset(vpp[:, :, :, DV + 1 : VW], 0.0)

            # ---- building blocks ----
            def proj_qk(src8, dst, c, p):
                """q/k projection for part p, chunk c: 4 fp8 DoubleRow
                matmuls (256-deep each), then PSUM->bf16 copy w/ bias."""
                pp = ps.tile([128, 512], F32, name="prj_qk", tag="prj", bufs=2)
                for j2 in range(4):
                    nc.tensor.matmul(
                        pp,
                        wq8[:, j2, p, :, :],
                        src8[:, 2 * j2 : 2 * j2 + 2, c * 512 : (c + 1) * 512],
                        start=(j2 == 0),
                        stop=(j2 == 3),
                        perf_mode=DRM,
                    )
                nc.scalar.activation(
                    dst[p][:, c * 512 : (c + 1) * 512],
                    pp,
                    mybir.ActivationFunctionType.Identity,
                    bias=bq_pp[:, p : p + 1],
                )

            def proj_v(m):
                """v projection for key block m: fp8 DoubleRow with e5m2
                error-feedback residuals on both operands (3 chains:
                x8*w8 + x8*wr + rx*w8) reconstructs the product to ~bf16
                accuracy at fp8-DR speed. Weights carry a 32x scale,
                removed in the bias-add STT."""
                pv = ps.tile([128, 512], F32, name="prj_v", tag="prj", bufs=2)
                pvv = pv[:, 0:DHC]
                for j2 in range(4):
                    nc.tensor.matmul(
                        pvv, xv8[:, j2, m, :, :], wv8[:, j2, :, :],
                        start=(j2 == 0), stop=False, perf_mode=DRM,
                    )
                for j2 in range(4):
                    nc.tensor.matmul(
                        pvv, xv8[:, j2, m, :, :], wvr8[:, j2, :, :],
                        start=False, stop=False, perf_mode=DRM,
                    )
                for j2 in range(4):
                    nc.tensor.matmul(
                        pvv, xvr8[:, j2, m, :, :], wv8[:, j2, :, :],
                        start=False, stop=(j2 == 3), perf_mode=DRM,
                    )
                nc.vector.scalar_tensor_tensor(
                    out=vpp[:, m, :, 0:DV],
                    in0=pvv.rearrange("p (h d) -> p h d", h=HPC),
                    scalar=1.0 / 32.0,
                    in1=bq_bc.rearrange("p (h d) -> p h d", h=HPC),
                    op0=MUL,
                    op1=ADD,
                )

            y_stage = {}

            def outproj_t(c, t, act_copy=False):
                yp = ps.tile([128, 512], F32, name="yp", tag="prj", bufs=2)
                for p in range(2):
                    nc.tensor.matmul(
                        yp,
                        woT[:, p, t * 128 : (t + 1) * 128],
                        xaT[p][:, c * 512 : (c + 1) * 512],
                        start=(p == 0),
                        stop=(p == 1),
                    )
                if t == 0:
                    y_stage[c] = yout.tile(
                        [128, DT, 512], F16, name="y_sb", tag="ystage",
                        bufs=2,
                    )
                y_sb = y_stage[c]
                if act_copy:
                    nc.scalar.copy(y_sb[:, t, :], yp)
                else:
                    nc.vector.tensor_copy(y_sb[:, t, :], yp)
                if c == 2:
                    nc.sync.dma_start(
                        out=yT_d.ap()[
                            t * 128 : (t + 1) * 128,
                            c * 512 : (c + 1) * 512,
                        ].rearrange("(t p) n -> p t n", p=128),
                        in_=y_sb[:, t : t + 1, :],
                    )
                elif t % 2 == 1:
                    t0 = t - 1
                    nc.sync.dma_start(
                        out=yT_d.ap()[
                            t0 * 128 : (t0 + 2) * 128,
                            c * 512 : (c + 1) * 512,
                        ].rearrange("(t p) n -> p t n", p=128),
                        in_=y_sb[:, t0 : t0 + 2, :],
                    )

            pT_tiles = {}
            op_tiles = {}
            xan_tiles = {}
            tp_tiles = {}

            def s_exp_burst(c, h, fillers):
                """S matmuls + exp for unit (c, h). Diagonal blocks:
                DVE STT schraudolph with mask folded into the B tile.
                Full blocks: ACT exact exp."""
                hp, hr = h // 2, h % 2
                fi = list(fillers)
                for j in range(4 * c + 4):
                    off = max(0, (j - 4 * c) * 128)
                    w = 512 - off
                    sp = ps.tile([128, 512], F32, name="sp", tag="spx", bufs=5)
                    pT = work.tile(
                        [128, 512], BF, name="pT", tag="pT", bufs=70
                    )
                    nc.tensor.matmul(
                        sp[:, 0:w],
                        kT[hp][
                            hr * 64 : (hr + 1) * 64, j * 128 : (j + 1) * 128
                        ],
                        qT[hp][
                            hr * 64 : (hr + 1) * 64,
                            c * 512 + off : (c + 1) * 512,
                        ],
                        start=True,
                        stop=True,
                    )
                    if off or j >= 4 * c:
                        # diagonal block: schraudolph + causal mask via B
                        nc.vector.scalar_tensor_tensor(
                            out=pT.bitcast(I16)[:, off:512],
                            in0=sp[:, 0:w],
                            scalar=A16,
                            in1=b16[:, 0:w],
                            op0=MUL,
                            op1=ADD,
                        )
                    elif (j % 4 == 3 and not (c == 2 and h >= 2)) or (
                        c == 3 and j % 4 == 1
                    ):
                        # every 4th full block: schraudolph on DVE to
                        # offload the ACT exp stream
                        nc.vector.tensor_scalar(
                            pT.bitcast(I16), sp, A16, M16, MUL, ADD
                        )
                    else:
                        nc.scalar.activation(
                            pT, sp, EXP, scale=SEXP, bias=negc
                        )
                    pT_tiles[(c, h, j)] = pT
                    if fi:
                        fi.pop(0)()
                for f in fi:
                    f()

            def pv_seg(c, h, t):
                """One query-block segment of transposed PV for unit
                (c, h): accumulate [128 queries, VW] chained over key
                blocks j <= 4c+t. Moving is the 66-wide v tile."""
                if t == 0:
                    op_tiles[(c, h)] = ps.tile(
                        [128, 4, VW], F32, name="op", tag="op", bufs=1
                    )
                op = op_tiles[(c, h)]
                qb = 4 * c + t
                qoff = t * 128
                for j in range(qb + 1):
                    pT = pT_tiles[(c, h, j)]
                    nc.tensor.matmul(
                        op[:, t, :],
                        pT[:, qoff : qoff + 128],
                        vpp[:, j, h, :],
                        start=(t == 0 and j == 0),
                        stop=(t == 3 and j == qb),
                        skip_group_check=True,
                    )
                if t == 3:
                    for j in range(4 * c + 4):
                        del pT_tiles[(c, h, j)]

            def norm_unit(c, h):
                """Normalize unit (c, h): per-partition reciprocal of the
                ones-column, one fused multiply."""
                op = op_tiles.pop((c, h))
                rcp = norm.tile([128, 4], F32, name="rcp", tag="rcp")
                nc.vector.reciprocal(rcp, op[:, :, DV])
                xan = norm.tile([128, 4, DV], BF, name="xan", tag="xan")
                nc.vector.scalar_tensor_tensor(
                    out=xan,
                    in0=op[:, :, 0:DV],
                    scalar=1.0,
                    in1=rcp.unsqueeze(2).broadcast_to([128, 4, DV]),
                    op0=MUL,
                    op1=MUL,
                )
                xan_tiles[(c, h)] = xan

            def transpose_xa(c, h):
                """PE-transpose xa [queries, dv] -> [dv, queries] into the
                shared tp psum tile; after the second head of a pair, copy
                the [128, 512] block out to xaT."""
                hp, hr = h // 2, h % 2
                xan = xan_tiles.pop((c, h))
                if hr == 0:
                    tp = ps.tile([128, 1024], BF, name="tp", tag="spx", bufs=5)
                    tp_tiles[(c, hp)] = tp
                tp = tp_tiles[(c, hp)]
                for t in range(4):
                    nc.tensor.matmul(
                        tp[hr * 64 : (hr + 1) * 64, t * 128 : (t + 1) * 128],
                        xan[:, t, :],
                        ident,
                        start=(hr == 0 and t == 0),
                        stop=(hr == 1 and t == 3),
                        is_transpose=True,
                        skip_group_check=True,
                    )
                if hr == 1:
                    del tp_tiles[(c, hp)]
                    nc.vector.tensor_copy(
                        xaT[hp][:, c * 512 : (c + 1) * 512], tp[:, 0:512]
                    )

            def F(fn, *a):
                return lambda: fn(*a)

            # ---- schedule: heavy (c=3/2) and light (c=0/1) units
            # interleaved so the exp engines' backlog from a heavy burst
            # drains while PE runs a light one ----
            UNITS = [
                (0, 0), (0, 1), (0, 2), (0, 3), (1, 0), (1, 1), (1, 2),
                (1, 3), (3, 0), (3, 1), (3, 2), (3, 3), (2, 0), (2, 1),
                (2, 2), (2, 3),
            ]
            fillers = {u: [] for u in UNITS}
            pre = {u: [] for u in UNITS}
            pre[(1, 0)] = [
                F(proj_qk, xkT8, kT, 1, 0),
                F(proj_qk, xkT8, kT, 1, 1),
                F(proj_qk, xqT8, qT, 1, 0),
                F(proj_qk, xqT8, qT, 1, 1),
            ]
            fillers[(0, 2)] = [F(proj_v, 0), F(proj_v, 1)]
            pre[(0, 3)] = [F(proj_v, 2), F(proj_v, 3)]
            fillers[(1, 2)] = [F(proj_v, m) for m in range(4, 8)]
            pre[(3, 0)] = [
                F(proj_qk, xqT8, qT, 3, 0),
                F(proj_qk, xqT8, qT, 3, 1),
                F(proj_qk, xkT8, kT, 3, 0),
                F(proj_qk, xkT8, kT, 3, 1),
            ]
            fillers[(3, 0)] = [
                F(proj_qk, xkT8, kT, 2, 0),
                F(proj_qk, xkT8, kT, 2, 1),
            ]
            fillers[(3, 1)] = [F(proj_v, m) for m in range(12, 16)] + [
                F(outproj_t, 0, t, bool(t % 2)) for t in range(4)
            ]
            fillers[(3, 2)] = [F(outproj_t, 0, t, bool(t % 2)) for t in range(4, 8)] + [
                F(proj_v, m) for m in range(8, 12)
            ]
            fillers[(3, 3)] = [
                F(proj_qk, xqT8, qT, 2, 0),
                F(proj_qk, xqT8, qT, 2, 1),
            ]
            fillers[(2, 0)] = [F(outproj_t, 1, t, bool(t % 2)) for t in range(4)]
            fillers[(2, 1)] = [F(outproj_t, 1, t, bool(t % 2)) for t in range(4, 8)]
            fillers[(2, 3)] = [F(outproj_t, 3, t, bool(t % 2)) for t in range(8)]

            # prologue: chunk-0 q/k projections
            for p in range(2):
                proj_qk(xkT8, kT, 0, p)
            for p in range(2):
                proj_qk(xqT8, qT, 0, p)

            for i, u in enumerate(UNITS):
                for f in pre[u]:
                    f()
                q = []
                if i >= 3:
                    u2 = UNITS[i - 3]
                    q += [F(pv_seg, *u2, t) for t in range(2)]
                if i >= 4:
                    q.append(F(transpose_xa, *UNITS[i - 4]))
                if i >= 3:
                    u2 = UNITS[i - 3]
                    q += [F(pv_seg, *u2, t) for t in range(2, 4)]
                    q.append(F(norm_unit, *u2))
                if i == 15:
                    u2 = UNITS[13]
                    q += [F(pv_seg, *u2, t) for t in range(4)]
                    q.append(F(norm_unit, *u2))
                q += fillers[u]
                s_exp_burst(*u, q)
            for i in (14, 15):
                u2 = UNITS[i]
                for t in range(4):
                    pv_seg(*u2, t)
                norm_unit(*u2)
            for i in (12, 13, 14, 15):
                transpose_xa(*UNITS[i])
            for t in range(DT):
                outproj_t(2, t, act_copy=bool(t % 2 == 0))
    nc.compile()
    return nc


def kernel(**inputs):
    inputs = {k: np.asarray(v) for k, v in inputs.items()}
    Q, K, V = inputs["Q"], inputs["K"], inputs["V"]
    wq, bq, wo, bo = inputs["wq"], inputs["bq"], inputs["wo"], inputs["bo"]
    f8 = ml_dtypes.float8_e4m3
    b16t = ml_dtypes.bfloat16

    def f8T(x):  # fp8 transpose [n, d] -> [d, n]
        return np.ascontiguousarray(x.astype(f8).T)

    def bfT(x):
        return np.ascontiguousarray(x.astype(b16t).T)

    xqT8 = [f8T(Q[b]) for b in range(B)]
    xkT8 = [f8T(K[b]) for b in range(B)]
    f8e5 = ml_dtypes.float8_e5m2
    xv8s, xvr8s = [], []
    for b in range(B):
        xvg = np.ascontiguousarray(V[b].T.astype(np.float32))  # [D, N]
        arr = xvg.reshape(4, 2, 128, NT, 128).transpose(2, 0, 3, 1, 4)
        a8 = arr.astype(f8)
        xv8s.append(np.ascontiguousarray(a8))
        xvr8s.append(
            np.ascontiguousarray((arr - a8.astype(np.float32)).astype(f8e5))
        )

    wq8s, wv8s, wvr8s, woTs, bq32s, bq1s = [], [], [], [], [], []
    for g in range(4):
        wqg = wq[g * DHC : (g + 1) * DHC, :]  # [256, 1024]
        # [p, j2, part, i, col]: value = 32*wq[part*128+col, (2*j2+i)*128+p]
        t = (32.0 * wqg).reshape(2, 128, 4, 2, 128)  # [part, col, j2, i, p]
        wq8s.append(
            np.ascontiguousarray(t.transpose(4, 2, 0, 3, 1).astype(f8))
        )
        wv = (32.0 * wqg).T.astype(np.float32)  # [D, 256]
        warr = wv.reshape(4, 2, 128, DHC).transpose(2, 0, 1, 3)
        w8v = warr.astype(f8)
        wv8s.append(np.ascontiguousarray(w8v))
        wvr8s.append(
            np.ascontiguousarray((warr - w8v.astype(np.float32)).astype(f8e5))
        )
        woTs.append(bfT(wo[:, g * DHC : (g + 1) * DHC]))
        bq32s.append(
            np.ascontiguousarray(
                32.0 * bq[g * DHC : (g + 1) * DHC], dtype=np.float32
            )
        )
        bq1s.append(
            np.ascontiguousarray(bq[g * DHC : (g + 1) * DHC], dtype=np.float32)
        )

    r = np.arange(128)[:, None]
    u = np.arange(512)[None, :]
    b16m = np.where(u >= r, np.float32(M16), np.float32(-1e9)).astype(
        np.float32
    )
    ident = np.eye(128, dtype=b16t)

    if "nc" not in _CACHE:
        _CACHE["nc"] = build_nc()
    nc = _CACHE["nc"]

    in_maps = []
    for core in range(8):
        b, g = divmod(core, 4)
        in_maps.append(
            {
                "xqT8": xqT8[b],
                "xkT8": xkT8[b],
                "xv8": xv8s[b],
                "xvr8": xvr8s[b],
                "wq8": wq8s[g],
                "wv8": wv8s[g],
                "wvr8": wvr8s[g],
                "woT": woTs[g],
                "bq32": bq32s[g],
                "bq1": bq1s[g],
                "b16": b16m,
                "ident": ident,
            }
        )
    import os

    trace = bool(int(os.environ.get("KERNEL_TRACE", "0")))
    try:
        res = run_bass_kernel_spmd(
            nc, in_maps, core_ids=list(range(8)), trace=trace
        )
    except ModuleNotFoundError:
        res = run_bass_kernel_spmd(nc, in_maps, core_ids=list(range(8)))
    _CACHE["last_results"] = res

    out = np.empty((B, N, D), np.float32)
    for b in range(B):
        acc = res.results[4 * b]["yT"].astype(np.float32)
        for g in range(1, 4):
            acc += res.results[4 * b + g]["yT"]
        out[b] = acc.T + bo
    return out
